# revision 5
# baseline (speedup 1.0000x reference)
"""CAM (channel attention module) Bass kernel for Trainium2.

Problem: y = gamma * (softmax_rev(v @ v.T * s) @ v) + x per batch sample,
with x [16, 128, 128, 128] f32, v = x.reshape(B, C, H*W).

Sharding: pure data parallel — B=16 split as 2 samples per core across
8 NeuronCores; gamma replicated; no collectives.

Per-core dataflow (per sample, [C=128, HW=16384]):
  1. DMA x sample into SBUF (f32, 4 quarter-loads for early start).
  2. Gram matrix E = V V^T via 128 chunk matmuls: PE transposes each f32
     [128,128] chunk (is_transpose matmul), ACT/DVE copy-casts PSUM->SBUF
     bf16 vT tile, PE accumulates vT.T @ vT into a PSUM bank (bf16 inputs,
     f32 accumulate).
  3. Reversed softmax: rowmin of E (DVE), p = exp(-s*E + s*rowmin) with
     fused row-sum Z (single ACT op), r = 1/Z (DVE), fold gamma: S' =
     p * (gamma*r) per row, cast bf16.
  4. PE-transpose S' -> S'T; attention-out chunks: psum = S'T.T @ v_bf16
     (32 matmuls, N=512), rhs downcast on the fly (ACT).
  5. y chunk = psum + x chunk (DVE tensor_add, f32) -> batched 1MB DMA out.
"""

import numpy as np

B, C, H, W = 16, 128, 128, 128
HW = H * W
N_CORES = 8
B_PER = B // N_CORES  # 2 samples per core
SCALE = 1.0 / float(np.sqrt(np.float32(HW)))  # 1/128

NQ = 4  # x quarter-loads per sample
QF = HW // NQ  # 4096 f32 per quarter
ATT_N = 512  # attention matmul moving free dim (one PSUM bank)
OUT_BLK = 2048  # output DMA batch (1 MB per [128, 2048] f32 block)


def _emit_sample(nc, mybir, pools, x_d, y_d, b):
    """Emit the full pipeline for one sample b."""
    f32 = mybir.dt.float32
    bf16 = mybir.dt.bfloat16
    (consts, xpool, vt_pool, vb_pool, sm_pool, out_pool, ps_t, ps_g, ps_a,
     ident_f32, ident_bf16, gamma_sb) = pools

    # ---- load sample (4 quarter tiles so chunk compute starts early)
    xq = []
    for q in range(NQ):
        xt = xpool.tile([128, QF], f32, tag="xq")
        nc.sync.dma_start(out=xt, in_=x_d[b, :, q * QF : (q + 1) * QF])
        xq.append(xt)

    # ---- Gram matrix: E = sum_k vT_k.T @ vT_k  (PSUM f32 [i, j])
    eps = ps_g.tile([128, 128], f32)
    for k in range(128):
        xt = xq[k // 32]
        col = (k % 32) * 128
        pt = ps_t.tile([128, 128], f32, tag="pt")
        nc.tensor.matmul(
            pt, xt[:, col : col + 128], ident_f32, is_transpose=True,
        )
        vt = vt_pool.tile([128, 128], bf16)
        if k % 2 == 0:
            nc.scalar.copy(vt, pt)
        else:
            nc.vector.tensor_copy(vt, pt)
        nc.tensor.matmul(
            eps, vt, vt, start=(k == 0), stop=(k == 127),
            skip_group_check=True,
        )

    # ---- reversed softmax with gamma/Z folded into rows
    rowmin = sm_pool.tile([128, 1], f32)
    nc.vector.tensor_reduce(
        rowmin, eps, axis=mybir.AxisListType.X, op=mybir.AluOpType.min
    )
    biasv = sm_pool.tile([128, 1], f32)
    nc.scalar.mul(biasv, rowmin, SCALE)
    p_sb = sm_pool.tile([128, 128], f32)
    zsum = sm_pool.tile([128, 1], f32)
    nc.scalar.activation(
        p_sb, eps, mybir.ActivationFunctionType.Exp,
        bias=biasv, scale=-SCALE, accum_out=zsum,
    )
    rz = sm_pool.tile([128, 1], f32)
    nc.vector.reciprocal(rz, zsum)
    rg = sm_pool.tile([128, 1], f32)
    nc.vector.tensor_mul(rg, rz, gamma_sb)
    sprime = sm_pool.tile([128, 128], bf16)
    nc.vector.tensor_scalar_mul(sprime, in0=p_sb, scalar1=rg)

    # transpose S' -> S'T (stationary for the attention matmuls)
    pst = ps_t.tile([128, 128], bf16, tag="pt")
    nc.tensor.matmul(pst, sprime, ident_bf16, is_transpose=True)
    spT = sm_pool.tile([128, 128], bf16)
    nc.vector.tensor_copy(spT, pst)

    # ---- attention out + residual, batched output DMA
    for j in range(HW // OUT_BLK):  # 8 blocks of 2048
        ot = out_pool.tile([128, OUT_BLK], f32)
        for tt in range(OUT_BLK // ATT_N):  # 4 chunks of 512
            t = j * (OUT_BLK // ATT_N) + tt
            xt = xq[t // 8]
            col = (t % 8) * ATT_N
            vb = vb_pool.tile([128, ATT_N], bf16)
            nc.scalar.copy(vb, xt[:, col : col + ATT_N])
            pa = ps_a.tile([128, ATT_N], f32)
            nc.tensor.matmul(pa, spT, vb)
            nc.vector.tensor_add(
                ot[:, tt * ATT_N : (tt + 1) * ATT_N],
                pa,
                xt[:, col : col + ATT_N],
            )
        nc.sync.dma_start(
            out=y_d[b, :, j * OUT_BLK : (j + 1) * OUT_BLK], in_=ot
        )


def _build_bass(reps=0):
    """Build the Bass program. reps>0 wraps the workload in a HW loop that
    repeats it (for steady-state benchmarking; output is idempotent)."""
    import concourse.bacc as bacc
    import concourse.tile as tile
    from concourse import masks, mybir
    from contextlib import ExitStack

    f32 = mybir.dt.float32
    bf16 = mybir.dt.bfloat16

    # Bacc (not plain Bass): its compile() runs generate_event_semaphores,
    # which splits multi-wait instructions — walrus rejects them on TRN2.
    nc = bacc.Bacc(
        "TRN2",
        target_bir_lowering=False,
        debug=False,
        enable_asserts=False,
        num_devices=N_CORES,
    )
    x_d = nc.dram_tensor("x", [B_PER, C, HW], f32, kind="ExternalInput")
    g_d = nc.dram_tensor("gamma", [1], f32, kind="ExternalInput")
    y_d = nc.dram_tensor("y", [B_PER, C, HW], f32, kind="ExternalOutput")

    with tile.TileContext(nc) as tc, ExitStack() as ctx:
        consts = ctx.enter_context(tc.tile_pool(name="consts", bufs=1))
        xpool = ctx.enter_context(tc.tile_pool(name="xpool", bufs=2 * NQ))
        vt_pool = ctx.enter_context(tc.tile_pool(name="vt", bufs=6))
        vb_pool = ctx.enter_context(tc.tile_pool(name="vb", bufs=3))
        sm_pool = ctx.enter_context(tc.tile_pool(name="sm", bufs=2))
        out_pool = ctx.enter_context(tc.tile_pool(name="outp", bufs=3))
        ps_t = ctx.enter_context(tc.tile_pool(name="ps_t", bufs=3, space="PSUM"))
        ps_g = ctx.enter_context(tc.tile_pool(name="ps_g", bufs=2, space="PSUM"))
        ps_a = ctx.enter_context(tc.tile_pool(name="ps_a", bufs=2, space="PSUM"))

        ident_f32 = consts.tile([128, 128], f32)
        masks.make_identity(nc, ident_f32)
        ident_bf16 = consts.tile([128, 128], bf16)
        masks.make_identity(nc, ident_bf16)
        gamma_sb = consts.tile([128, 1], f32)
        nc.gpsimd.dma_start(out=gamma_sb, in_=g_d[:].to_broadcast((128, 1)))

        pools = (consts, xpool, vt_pool, vb_pool, sm_pool, out_pool,
                 ps_t, ps_g, ps_a, ident_f32, ident_bf16, gamma_sb)

        if reps:
            with tc.For_i(0, reps, 1):
                for b in range(B_PER):
                    _emit_sample(nc, mybir, pools, x_d, y_d, b)
        else:
            for b in range(B_PER):
                _emit_sample(nc, mybir, pools, x_d, y_d, b)

    nc.compile()
    return nc


_NC_CACHE = None


def _get_nc():
    global _NC_CACHE
    if _NC_CACHE is None:
        _NC_CACHE = _build_bass()
    return _NC_CACHE


def kernel(x, gamma, trace=False):
    from concourse.bass_utils import run_bass_kernel_spmd

    x = np.asarray(x, dtype=np.float32)
    gamma = np.asarray(gamma, dtype=np.float32)
    nc = _get_nc()

    xs = x.reshape(N_CORES, B_PER, C, HW)
    in_maps = [{"x": xs[i], "gamma": gamma} for i in range(N_CORES)]
    res = run_bass_kernel_spmd(nc, in_maps, core_ids=list(range(N_CORES)), trace=trace)
    out = np.stack([res.results[i]["y"] for i in range(N_CORES)], axis=0)
    out = out.reshape(B, C, H, W)
    if trace:
        return out, res
    return out


# revision 10
# speedup vs baseline: 1.2136x; 1.2136x over previous
"""CAM (channel attention module) Bass kernel for Trainium2.

Problem: y = gamma * (softmax_rev(v @ v.T * s) @ v) + x per batch sample,
with x [16, 128, 128, 128] f32, v = x.reshape(B, C, H*W).

Sharding: pure data parallel — B=16 split as 2 samples per core across
8 NeuronCores; gamma replicated; no collectives.

Per-core dataflow (per sample, [C=128, HW=16384]):
  1. DMA both samples into SBUF up front (f32 quarter-loads) so the input
     stream never stalls behind output DMAs.
  2. Gram matrix E = V V^T: PE transposes f32 chunks (4 per PSUM bank),
     one ACT copy-cast PSUM->SBUF bf16 per group, PE accumulates
     vT.T @ vT into a PSUM bank (bf16 inputs, f32 accumulate).
  3. Reversed softmax: rowmin of E (DVE), p = exp(-s*E + s*rowmin) with
     fused row-sum Z (single ACT op), r = 1/Z (DVE), fold gamma: S' =
     p * (gamma*r) per row; PE-transpose -> bf16 stationary S'T.
  4. Attention: psum = S'T.T @ v_bf16 (32 matmuls, N=512); rhs bf16
     copies alternate between GPSIMD and ACT so neither paces the loop;
     y chunk = psum + x chunk (DVE f32 add) -> batched 1MB DMA out.
  Sample 0's attention phase is interleaved with sample 1's Gram phase in
  emission order so the PE/ACT streams of the two samples overlap.
"""

import numpy as np

B, C, H, W = 16, 128, 128, 128
HW = H * W
N_CORES = 8
B_PER = B // N_CORES  # 2 samples per core
SCALE = 1.0 / float(np.sqrt(np.float32(HW)))  # 1/128

NQ = 4  # x quarter-loads per sample
QF = HW // NQ  # 4096 f32 per quarter
ATT_N = 512  # attention matmul moving free dim (one PSUM bank)
OUT_BLK = 2048  # output DMA batch (1 MB per [128, 2048] f32 block)
N_GROUPS = 32  # gram groups (4 transposed chunks each)
N_BLOCKS = HW // OUT_BLK  # 8 attention/output blocks


class _SampleCtx:
    """Per-sample tiles threaded between the emission phases."""

    def __init__(self):
        self.xq = None
        self.eps = None
        self.spT = None


def _emit_load(nc, mybir, pools, x_d, b, sc):
    f32 = mybir.dt.float32
    xpool = pools["xpool"]
    sc.xq = []
    for q in range(NQ):
        xt = xpool.tile([128, QF], f32, tag="xq")
        nc.sync.dma_start(out=xt, in_=x_d[b, :, q * QF : (q + 1) * QF])
        sc.xq.append(xt)


def _emit_gram_groups(nc, mybir, pools, sc, groups):
    """Gram accumulation for the given group indices (4 chunks per group)."""
    f32 = mybir.dt.float32
    bf16 = mybir.dt.bfloat16
    if sc.eps is None:
        sc.eps = pools["ps_g"].tile([128, 128], f32)
    for g in groups:
        xt = sc.xq[g // 8]
        gcol = (g % 8) * 512
        pt = pools["ps_t"].tile([128, 512], f32, tag="pt")
        for i in range(4):
            nc.tensor.matmul(
                pt[:, i * 128 : (i + 1) * 128],
                xt[:, gcol + i * 128 : gcol + (i + 1) * 128],
                pools["ident_f32"],
                is_transpose=True,
                skip_group_check=True,
            )
        vt = pools["vt"].tile([128, 512], bf16)
        nc.scalar.copy(vt, pt)  # ACT: PSUM f32 -> SBUF bf16
        for i in range(4):
            k = g * 4 + i
            vti = vt[:, i * 128 : (i + 1) * 128]
            nc.tensor.matmul(
                sc.eps, vti, vti, start=(k == 0), stop=(k == 127),
                skip_group_check=True,
            )


def _emit_softmax(nc, mybir, pools, sc):
    """Reversed softmax + gamma fold; produces bf16 stationary S'T."""
    f32 = mybir.dt.float32
    bf16 = mybir.dt.bfloat16
    sm_pool = pools["sm"]
    eps = sc.eps
    rowmin = sm_pool.tile([128, 1], f32)
    nc.vector.tensor_reduce(
        rowmin, eps, axis=mybir.AxisListType.X, op=mybir.AluOpType.min
    )
    biasv = sm_pool.tile([128, 1], f32)
    nc.scalar.mul(biasv, rowmin, SCALE)
    p_sb = sm_pool.tile([128, 128], f32)
    zsum = sm_pool.tile([128, 1], f32)
    nc.scalar.activation(
        p_sb, eps, mybir.ActivationFunctionType.Exp,
        bias=biasv, scale=-SCALE, accum_out=zsum,
    )
    rz = sm_pool.tile([128, 1], f32)
    nc.vector.reciprocal(rz, zsum)
    rg = sm_pool.tile([128, 1], f32)
    nc.vector.tensor_mul(rg, rz, pools["gamma_sb"])
    sprime = sm_pool.tile([128, 128], f32)
    nc.vector.tensor_scalar_mul(sprime, in0=p_sb, scalar1=rg)

    pst = pools["ps_t"].tile([128, 512], f32, tag="pt")
    nc.tensor.matmul(pst[:, 0:128], sprime, pools["ident_f32"],
                     is_transpose=True, skip_group_check=True)
    spT = sm_pool.tile([128, 128], bf16)
    nc.vector.tensor_copy(spT, pst[:, 0:128])
    sc.spT = spT


def _emit_attn_block(nc, mybir, pools, y_d, b, sc, j):
    """One [128, OUT_BLK] attention+residual block + output DMA."""
    f32 = mybir.dt.float32
    bf16 = mybir.dt.bfloat16
    ot = pools["outp"].tile([128, OUT_BLK], f32)
    for tt in range(OUT_BLK // ATT_N):
        t = j * (OUT_BLK // ATT_N) + tt
        xt = sc.xq[t // 8]
        col = (t % 8) * ATT_N
        vb = pools["vb"].tile([128, ATT_N], bf16)
        # alternate producers so neither engine paces the attention loop
        if tt % 2 == 0:
            nc.gpsimd.tensor_copy(vb, xt[:, col : col + ATT_N])
        else:
            nc.scalar.copy(vb, xt[:, col : col + ATT_N])
        pa = pools["ps_a"].tile([128, ATT_N], f32)
        nc.tensor.matmul(pa, sc.spT, vb)
        nc.vector.tensor_add(
            ot[:, tt * ATT_N : (tt + 1) * ATT_N],
            pa,
            xt[:, col : col + ATT_N],
        )
    nc.sync.dma_start(out=y_d[b, :, j * OUT_BLK : (j + 1) * OUT_BLK], in_=ot)


def _emit_workload(nc, mybir, pools, x_d, y_d):
    """Both samples, software-pipelined in emission order."""
    s0, s1 = _SampleCtx(), _SampleCtx()
    _emit_load(nc, mybir, pools, x_d, 0, s0)
    _emit_load(nc, mybir, pools, x_d, 1, s1)

    _emit_gram_groups(nc, mybir, pools, s0, range(N_GROUPS))
    _emit_softmax(nc, mybir, pools, s0)

    # interleave: sample-0 attention blocks with sample-1 gram groups
    gper = N_GROUPS // N_BLOCKS  # 4 groups per block
    for j in range(N_BLOCKS):
        _emit_attn_block(nc, mybir, pools, y_d, 0, s0, j)
        _emit_gram_groups(nc, mybir, pools, s1, range(j * gper, (j + 1) * gper))

    _emit_softmax(nc, mybir, pools, s1)
    for j in range(N_BLOCKS):
        _emit_attn_block(nc, mybir, pools, y_d, 1, s1, j)


def _build_bass(reps=0):
    """Build the Bass program. reps>0 wraps the workload in a HW loop that
    repeats it (for steady-state benchmarking; output is idempotent)."""
    import concourse.bacc as bacc
    import concourse.tile as tile
    from concourse import masks, mybir
    from contextlib import ExitStack

    f32 = mybir.dt.float32

    # Bacc (not plain Bass): its compile() runs generate_event_semaphores,
    # which splits multi-wait instructions — walrus rejects them on TRN2.
    nc = bacc.Bacc(
        "TRN2",
        target_bir_lowering=False,
        debug=False,
        enable_asserts=False,
        num_devices=N_CORES,
    )
    x_d = nc.dram_tensor("x", [B_PER, C, HW], f32, kind="ExternalInput")
    g_d = nc.dram_tensor("gamma", [1], f32, kind="ExternalInput")
    y_d = nc.dram_tensor("y", [B_PER, C, HW], f32, kind="ExternalOutput")

    with tile.TileContext(nc) as tc, ExitStack() as ctx:
        pools = {}
        for name, kw in [
            ("consts", dict(bufs=1)),
            ("xpool", dict(bufs=2 * NQ)),
            ("vt", dict(bufs=3)),
            ("vb", dict(bufs=4)),
            ("sm", dict(bufs=2)),
            ("outp", dict(bufs=3)),
            ("ps_t", dict(bufs=3, space="PSUM")),
            ("ps_g", dict(bufs=2, space="PSUM")),
            ("ps_a", dict(bufs=3, space="PSUM")),
        ]:
            pools[name] = ctx.enter_context(tc.tile_pool(name=name, **kw))

        ident_f32 = pools["consts"].tile([128, 128], f32)
        masks.make_identity(nc, ident_f32)
        gamma_sb = pools["consts"].tile([128, 1], f32)
        nc.gpsimd.dma_start(out=gamma_sb, in_=g_d[:].to_broadcast((128, 1)))
        pools["ident_f32"] = ident_f32
        pools["gamma_sb"] = gamma_sb

        if reps:
            with tc.For_i(0, reps, 1):
                _emit_workload(nc, mybir, pools, x_d, y_d)
        else:
            _emit_workload(nc, mybir, pools, x_d, y_d)

    nc.compile()
    return nc


_NC_CACHE = None


def _get_nc():
    global _NC_CACHE
    if _NC_CACHE is None:
        _NC_CACHE = _build_bass()
    return _NC_CACHE


def kernel(x, gamma, trace=False):
    from concourse.bass_utils import run_bass_kernel_spmd

    x = np.asarray(x, dtype=np.float32)
    gamma = np.asarray(gamma, dtype=np.float32)
    nc = _get_nc()

    xs = x.reshape(N_CORES, B_PER, C, HW)
    in_maps = [{"x": xs[i], "gamma": gamma} for i in range(N_CORES)]
    res = run_bass_kernel_spmd(nc, in_maps, core_ids=list(range(N_CORES)), trace=trace)
    out = np.stack([res.results[i]["y"] for i in range(N_CORES)], axis=0)
    out = out.reshape(B, C, H, W)
    if trace:
        return out, res
    return out
